# revision 2
# baseline (speedup 1.0000x reference)
"""Trainium2 Bass kernel for nn_MemoryBank (vq_codebook softmax).

C[b, s, t] = softmax_s(-||H[b,:,t] - units[:,s]||^2)
           = softmax_s(cross'[t,s] - m_sq[s]),  cross' = H[b].T @ (2*units)

Strategy (8 NeuronCores, data-parallel over batch B=64 -> 8 per core):
  - Layout: t on PARTITIONS (chunks of 128), s on the free axis (1024).
    Softmax over s becomes a native DVE free-axis reduce; the shift is an
    exact per-partition fp32 ACT bias. No GPSIMD, no rank-1 broadcast
    matmuls, no ones-matmul denominators: the PE does ONLY the GEMM.
  - bf16 3-term split GEMM (h1u1 + h1u2 + h2u1) for ~fp32-accurate logits,
    accumulated in PSUM ([128, 1024] fp32 = 2 banks per chunk).
  - Per 128-t chunk:
      ttr:  l = cr + (-m_sq) broadcast tile, fused max-reduce -> mx
      exp1: ACT Exp(l - max) -> fp16 scratch, accum_out = den (fp32)
      ln:   ACT Ln(den); bias2 = -(max + ln den)  (exact fp32)
      exp2: ACT Exp(l + bias2) -> fp32 out tile -> DMA (t-chunk-major)
  - Host transposes [b, nt, 128, s] -> (B, S, T) at the end.
"""
import numpy as np
import ml_dtypes

import concourse.bacc as bacc
import concourse.bass as bass
import concourse.bass_isa as bass_isa
import concourse.mybir as mybir
import concourse.tile as tile

F32 = mybir.dt.float32
BF16 = mybir.dt.bfloat16
FP16 = mybir.dt.float16
AF = mybir.ActivationFunctionType
ALU = mybir.AluOpType
AX = mybir.AxisListType

# Problem shape (hardcoded per harness contract)
B, D, T, S = 64, 512, 2048, 1024
NCORES = 8
B_SH = B // NCORES          # batches per core
DC = D // 128               # d chunks of 128
NT = T // 128               # t chunks of 128 (partition dim of compute)
NEG_INF = -3.0e38


def build_kernel(b_sh=B_SH):
    nc = bacc.Bacc(None, target_bir_lowering=False, debug=False)

    h1_d = nc.dram_tensor("h1", [b_sh, DC, 128, T], BF16, kind="ExternalInput")
    h2_d = nc.dram_tensor("h2", [b_sh, DC, 128, T], BF16, kind="ExternalInput")
    u1_d = nc.dram_tensor("u1", [DC, 128, S], BF16, kind="ExternalInput")
    u2_d = nc.dram_tensor("u2", [DC, 128, S], BF16, kind="ExternalInput")
    msq_d = nc.dram_tensor("msq", [128, S], F32, kind="ExternalInput")
    c_d = nc.dram_tensor("C", [b_sh, NT, 128, S], F32, kind="ExternalOutput")

    with tile.TileContext(nc) as tc:
        with (
            tc.tile_pool(name="const", bufs=1) as cpool,
            tc.tile_pool(name="hbuf", bufs=2) as hpool,
            tc.tile_pool(name="lg", bufs=3) as lpool,
            tc.tile_pool(name="ex", bufs=2) as epool,
            tc.tile_pool(name="outp", bufs=3) as opool,
            tc.tile_pool(name="st", bufs=4) as spool,
            tc.tile_pool(name="ps", bufs=3, space="PSUM") as ps,
        ):
            # --- constants loaded once ---
            u1_sb = cpool.tile([128, DC, S], BF16, tag="u1")
            u2_sb = cpool.tile([128, DC, S], BF16, tag="u2")
            nc.sync.dma_start(u1_sb[:], u1_d.rearrange("c p s -> p c s"))
            nc.sync.dma_start(u2_sb[:], u2_d.rearrange("c p s -> p c s"))
            msq_sb = cpool.tile([128, S], F32, tag="msq")
            nc.sync.dma_start(msq_sb[:], msq_d[:])

            for b in range(b_sh):
                h1_sb = hpool.tile([128, DC, T], BF16, tag="h1")
                h2_sb = hpool.tile([128, DC, T], BF16, tag="h2")
                nc.sync.dma_start(h1_sb[:], h1_d[b].rearrange("c p t -> p c t"))
                nc.sync.dma_start(h2_sb[:], h2_d[b].rearrange("c p t -> p c t"))

                for it in range(NT):
                    t0 = it * 128
                    cr = ps.tile([128, S], F32, tag="cr")
                    mms = [(u1_sb, h1_sb), (u1_sb, h2_sb), (u2_sb, h1_sb)]
                    n = DC * len(mms)
                    i = 0
                    for c in range(DC):
                        for (uu, hh) in mms:
                            nc.tensor.matmul(
                                cr[:],
                                hh[:, c, t0:t0 + 128],
                                uu[:, c, :],
                                start=(i == 0), stop=(i == n - 1),
                            )
                            i += 1

                    # l = cr - m_sq (fused with max-reduce)
                    l = lpool.tile([128, S], F32, tag="l")
                    mx = spool.tile([128, 1], F32, tag="mx")
                    nc.vector.tensor_tensor_reduce(
                        l[:], cr[:], msq_sb[:], 1.0, NEG_INF,
                        op0=ALU.add, op1=ALU.max, accum_out=mx[:],
                    )
                    mxn = spool.tile([128, 1], F32, tag="mxn")
                    nc.vector.tensor_scalar_mul(mxn[:], mx[:], -1.0)

                    # exp1 + denominator in one ACT pass
                    e1 = epool.tile([128, S], FP16, tag="e1")
                    den = spool.tile([128, 1], F32, tag="den")
                    nc.scalar.activation(
                        e1[:], l[:], AF.Exp, bias=mxn[:], scale=1.0,
                        accum_out=den[:],
                    )
                    lnd = spool.tile([128, 1], F32, tag="lnd")
                    nc.scalar.activation(lnd[:], den[:], AF.Ln)
                    b2 = spool.tile([128, 1], F32, tag="b2")
                    nc.vector.scalar_tensor_tensor(
                        b2[:], lnd[:], -1.0, mxn[:],
                        op0=ALU.mult, op1=ALU.add,
                    )

                    # final normalized probabilities
                    o = opool.tile([128, S], F32, tag="o")
                    nc.scalar.activation(
                        o[:], l[:], AF.Exp, bias=b2[:], scale=1.0)
                    nc.sync.dma_start(c_d[b, it], o[:])

    nc.compile()
    return nc


# ---------------------------------------------------------------- host side

_RUNNER = None


def _get_runner():
    global _RUNNER
    if _RUNNER is None:
        nc = build_kernel()
        _RUNNER = _BassPjrtRunner(nc, NCORES)
    return _RUNNER


def _split_bf16(x):
    hi = x.astype(ml_dtypes.bfloat16)
    lo = (x - hi.astype(np.float32)).astype(ml_dtypes.bfloat16)
    return hi, lo


def prep_inputs(H, units):
    H = np.ascontiguousarray(np.asarray(H, dtype=np.float32))
    U = np.ascontiguousarray(np.asarray(units, dtype=np.float32))
    h1, h2 = _split_bf16(H)
    u1, u2 = _split_bf16(2.0 * U)
    msq = -(U.astype(np.float64) ** 2).sum(0).astype(np.float32)
    msq_b = np.ascontiguousarray(np.broadcast_to(msq[None, :], (128, S)))

    u1 = u1.reshape(DC, 128, S)
    u2 = u2.reshape(DC, 128, S)
    in_maps = []
    for c in range(NCORES):
        sl = slice(c * B_SH, (c + 1) * B_SH)
        in_maps.append({
            "h1": h1[sl].reshape(B_SH, DC, 128, T),
            "h2": h2[sl].reshape(B_SH, DC, 128, T),
            "u1": u1, "u2": u2, "msq": msq_b,
        })
    return in_maps


def kernel(H, units):
    runner = _get_runner()
    in_maps = prep_inputs(H, units)
    args = runner.prep_inputs(in_maps)
    outs = runner.run(args)
    c = np.asarray(outs[0])           # (NCORES*B_SH, NT, 128, S)
    c = c.reshape(B, NT, 128, S)
    return np.ascontiguousarray(c.transpose(0, 3, 1, 2)).reshape(B, S, T)


# ------------------------------------------------- embedded PJRT runner

class _BassPjrtRunner:
    def __init__(self, nc, n_cores):
        import jax
        from jax.sharding import Mesh, PartitionSpec
        from jax.experimental.shard_map import shard_map
        from concourse import bass2jax

        bass2jax.install_neuronx_cc_hook()
        self.n_cores = n_cores
        partition_name = (
            nc.partition_id_tensor.name if nc.partition_id_tensor else None
        )
        in_names, out_names, out_avals, zero_outs = [], [], [], []
        for alloc in nc.m.functions[0].allocations:
            if not isinstance(alloc, mybir.MemoryLocationSet):
                continue
            name = alloc.memorylocations[0].name
            if alloc.kind == "ExternalInput":
                if name != partition_name:
                    in_names.append(name)
            elif alloc.kind == "ExternalOutput":
                shape = tuple(alloc.tensor_shape)
                dtype = mybir.dt.np(alloc.dtype)
                out_names.append(name)
                out_avals.append(jax.core.ShapedArray(shape, dtype))
                zero_outs.append((shape, dtype))
        self.in_names = in_names
        self.out_names = out_names
        self.out_shapes = zero_outs
        n_params = len(in_names)
        n_outs = len(out_avals)
        all_in_names = in_names + out_names
        if partition_name is not None:
            all_in_names.append(partition_name)
        self.n_params = n_params

        def _body(*args):
            operands = list(args)
            if partition_name is not None:
                operands.append(bass2jax.partition_id_tensor())
            outs = bass2jax._bass_exec_p.bind(
                *operands,
                out_avals=tuple(out_avals),
                in_names=tuple(all_in_names),
                out_names=tuple(out_names),
                lowering_input_output_aliases=(),
                sim_require_finite=False,
                sim_require_nnan=False,
                nc=nc,
            )
            return tuple(outs)

        devices = jax.devices()[:n_cores]
        assert len(devices) == n_cores
        if n_cores == 1:
            self._fn = jax.jit(_body, keep_unused=True)
        else:
            mesh = Mesh(np.asarray(devices), ("core",))
            in_specs = (PartitionSpec("core"),) * (n_params + n_outs)
            out_specs = (PartitionSpec("core"),) * n_outs
            self._fn = jax.jit(
                shard_map(_body, mesh=mesh, in_specs=in_specs,
                          out_specs=out_specs, check_rep=False),
                keep_unused=True,
            )

    def prep_inputs(self, in_maps):
        per_core = [[np.asarray(m[n]) for n in self.in_names] for m in in_maps]
        if self.n_cores == 1:
            args = per_core[0]
        else:
            args = [
                np.concatenate([per_core[c][i] for c in range(self.n_cores)], 0)
                for i in range(self.n_params)
            ]
        zouts = []
        for (s, d) in self.out_shapes:
            full = (s[0] * self.n_cores,) + tuple(s[1:]) \
                if self.n_cores > 1 else s
            zouts.append(np.zeros(full, d))
        return args + zouts

    def run(self, args):
        import jax
        outs = self._fn(*args)
        jax.block_until_ready(outs)
        return outs


# revision 4
# speedup vs baseline: 18122.6071x; 18122.6071x over previous
"""Trainium2 Bass kernel for nn_MemoryBank (vq_codebook softmax).

C[b, s, t] = softmax_s(-||H[b,:,t] - units[:,s]||^2)
           = softmax_s(cross'[t,s] - m_sq[s]),  cross' = H[b].T @ (2*units)

Strategy (8 NeuronCores, data-parallel over batch B=64 -> 8 per core):
  - Layout: t on PARTITIONS (chunks of 128), s on the free axis (1024).
    Softmax over s becomes a native DVE free-axis reduce; the shift is an
    exact per-partition fp32 ACT bias. No GPSIMD, no rank-1 broadcast
    matmuls, no ones-matmul denominators: the PE does ONLY the GEMM.
  - bf16 3-term split GEMM (h1u1 + h1u2 + h2u1) for ~fp32-accurate logits,
    accumulated in PSUM ([128, 1024] fp32 = 2 banks per chunk).
  - Per 128-t chunk:
      ttr:  l = cr + (-m_sq) broadcast tile, fused max-reduce -> mx
      exp1: ACT Exp(l - max) -> fp16 scratch, accum_out = den (fp32)
      ln:   ACT Ln(den); bias2 = -(max + ln den)  (exact fp32)
      exp2: ACT Exp(l + bias2) -> fp32 out tile -> DMA (t-chunk-major)
  - Host transposes [b, nt, 128, s] -> (B, S, T) at the end.
"""
import numpy as np
import ml_dtypes

import concourse.bacc as bacc
import concourse.bass as bass
import concourse.bass_isa as bass_isa
import concourse.mybir as mybir
import concourse.tile as tile

F32 = mybir.dt.float32
BF16 = mybir.dt.bfloat16
FP16 = mybir.dt.float16
AF = mybir.ActivationFunctionType
ALU = mybir.AluOpType
AX = mybir.AxisListType

# Problem shape (hardcoded per harness contract)
B, D, T, S = 64, 512, 2048, 1024
NCORES = 8
B_SH = B // NCORES          # batches per core
DC = D // 128               # d chunks of 128
NT = T // 128               # t chunks of 128 (partition dim of compute)
NEG_INF = -3.0e38


def build_kernel(b_sh=B_SH):
    nc = bacc.Bacc(None, target_bir_lowering=False, debug=False)

    h1_d = nc.dram_tensor("h1", [b_sh, DC, 128, T], BF16, kind="ExternalInput")
    h2_d = nc.dram_tensor("h2", [b_sh, DC, 128, T], BF16, kind="ExternalInput")
    u1_d = nc.dram_tensor("u1", [DC, 128, S], BF16, kind="ExternalInput")
    u2_d = nc.dram_tensor("u2", [DC, 128, S], BF16, kind="ExternalInput")
    msq_d = nc.dram_tensor("msq", [128, S], F32, kind="ExternalInput")
    c_d = nc.dram_tensor("C", [b_sh, NT, 128, S], F32, kind="ExternalOutput")

    with tile.TileContext(nc) as tc:
        with (
            tc.tile_pool(name="const", bufs=1) as cpool,
            tc.tile_pool(name="hbuf", bufs=2) as hpool,
            tc.tile_pool(name="lg", bufs=3) as lpool,
            tc.tile_pool(name="ex", bufs=2) as epool,
            tc.tile_pool(name="outp", bufs=3) as opool,
            tc.tile_pool(name="st", bufs=4) as spool,
            tc.tile_pool(name="ps", bufs=3, space="PSUM") as ps,
        ):
            # --- constants loaded once ---
            u1_sb = cpool.tile([128, DC, S], BF16, tag="u1")
            u2_sb = cpool.tile([128, DC, S], BF16, tag="u2")
            nc.sync.dma_start(u1_sb[:], u1_d.rearrange("c p s -> p c s"))
            nc.sync.dma_start(u2_sb[:], u2_d.rearrange("c p s -> p c s"))
            msq_sb = cpool.tile([128, S], F32, tag="msq")
            nc.sync.dma_start(msq_sb[:], msq_d[:])

            for b in range(b_sh):
                h1_sb = hpool.tile([128, DC, T], BF16, tag="h1")
                h2_sb = hpool.tile([128, DC, T], BF16, tag="h2")
                nc.sync.dma_start(h1_sb[:], h1_d[b].rearrange("c p t -> p c t"))
                nc.sync.dma_start(h2_sb[:], h2_d[b].rearrange("c p t -> p c t"))

                for it in range(NT):
                    t0 = it * 128
                    cr = ps.tile([128, S], F32, tag="cr")
                    mms = [(u1_sb, h1_sb), (u1_sb, h2_sb), (u2_sb, h1_sb)]
                    n = DC * len(mms)
                    for half in range(2):
                        sl = slice(half * 512, (half + 1) * 512)
                        i = 0
                        for c in range(DC):
                            for (uu, hh) in mms:
                                nc.tensor.matmul(
                                    cr[:, sl],
                                    hh[:, c, t0:t0 + 128],
                                    uu[:, c, sl],
                                    start=(i == 0), stop=(i == n - 1),
                                )
                                i += 1

                    # l = cr - m_sq (fused with max-reduce)
                    l = lpool.tile([128, S], F32, tag="l")
                    mx = spool.tile([128, 1], F32, tag="mx")
                    nc.vector.tensor_tensor_reduce(
                        l[:], cr[:], msq_sb[:], 1.0, NEG_INF,
                        op0=ALU.add, op1=ALU.max, accum_out=mx[:],
                    )
                    mxn = spool.tile([128, 1], F32, tag="mxn")
                    nc.vector.tensor_scalar_mul(mxn[:], mx[:], -1.0)

                    # exp1 + denominator in one ACT pass
                    e1 = epool.tile([128, S], FP16, tag="e1")
                    den = spool.tile([128, 1], F32, tag="den")
                    nc.scalar.activation(
                        e1[:], l[:], AF.Exp, bias=mxn[:], scale=1.0,
                        accum_out=den[:],
                    )
                    lnd = spool.tile([128, 1], F32, tag="lnd")
                    nc.scalar.activation(lnd[:], den[:], AF.Ln)
                    b2 = spool.tile([128, 1], F32, tag="b2")
                    nc.vector.scalar_tensor_tensor(
                        b2[:], lnd[:], -1.0, mxn[:],
                        op0=ALU.mult, op1=ALU.add,
                    )

                    # final normalized probabilities
                    o = opool.tile([128, S], F32, tag="o")
                    nc.scalar.activation(
                        o[:], l[:], AF.Exp, bias=b2[:], scale=1.0)
                    nc.sync.dma_start(c_d[b, it], o[:])

    nc.compile()
    return nc


# ---------------------------------------------------------------- host side

_RUNNER = None


def _get_runner():
    global _RUNNER
    if _RUNNER is None:
        nc = build_kernel()
        _RUNNER = _BassPjrtRunner(nc, NCORES)
    return _RUNNER


def _split_bf16(x):
    hi = x.astype(ml_dtypes.bfloat16)
    lo = (x - hi.astype(np.float32)).astype(ml_dtypes.bfloat16)
    return hi, lo


def prep_inputs(H, units):
    H = np.ascontiguousarray(np.asarray(H, dtype=np.float32))
    U = np.ascontiguousarray(np.asarray(units, dtype=np.float32))
    h1, h2 = _split_bf16(H)
    u1, u2 = _split_bf16(2.0 * U)
    msq = -(U.astype(np.float64) ** 2).sum(0).astype(np.float32)
    msq_b = np.ascontiguousarray(np.broadcast_to(msq[None, :], (128, S)))

    u1 = u1.reshape(DC, 128, S)
    u2 = u2.reshape(DC, 128, S)
    in_maps = []
    for c in range(NCORES):
        sl = slice(c * B_SH, (c + 1) * B_SH)
        in_maps.append({
            "h1": h1[sl].reshape(B_SH, DC, 128, T),
            "h2": h2[sl].reshape(B_SH, DC, 128, T),
            "u1": u1, "u2": u2, "msq": msq_b,
        })
    return in_maps


def kernel(H, units):
    runner = _get_runner()
    in_maps = prep_inputs(H, units)
    args = runner.prep_inputs(in_maps)
    outs = runner.run(args)
    c = np.asarray(outs[0])           # (NCORES*B_SH, NT, 128, S)
    c = c.reshape(B, NT, 128, S)
    return np.ascontiguousarray(c.transpose(0, 3, 1, 2)).reshape(B, S, T)


# ------------------------------------------------- embedded PJRT runner

class _BassPjrtRunner:
    def __init__(self, nc, n_cores):
        import jax
        from jax.sharding import Mesh, PartitionSpec
        from jax.experimental.shard_map import shard_map
        from concourse import bass2jax

        bass2jax.install_neuronx_cc_hook()
        self.n_cores = n_cores
        self.nc = nc
        partition_name = (
            nc.partition_id_tensor.name if nc.partition_id_tensor else None
        )
        self.partition_name = partition_name
        in_names, out_names, out_avals, zero_outs = [], [], [], []
        for alloc in nc.m.functions[0].allocations:
            if not isinstance(alloc, mybir.MemoryLocationSet):
                continue
            name = alloc.memorylocations[0].name
            if alloc.kind == "ExternalInput":
                if name != partition_name:
                    in_names.append(name)
            elif alloc.kind == "ExternalOutput":
                shape = tuple(alloc.tensor_shape)
                dtype = mybir.dt.np(alloc.dtype)
                out_names.append(name)
                out_avals.append(jax.core.ShapedArray(shape, dtype))
                zero_outs.append((shape, dtype))
        self.in_names = in_names
        self.out_names = out_names
        self.out_shapes = zero_outs
        n_params = len(in_names)
        n_outs = len(out_avals)
        all_in_names = in_names + out_names
        if partition_name is not None:
            all_in_names.append(partition_name)
        self.n_params = n_params

        def _body(*args):
            operands = list(args)
            if partition_name is not None:
                operands.append(bass2jax.partition_id_tensor())
            outs = bass2jax._bass_exec_p.bind(
                *operands,
                out_avals=tuple(out_avals),
                in_names=tuple(all_in_names),
                out_names=tuple(out_names),
                lowering_input_output_aliases=(),
                sim_require_finite=False,
                sim_require_nnan=False,
                nc=nc,
            )
            return tuple(outs)

        devices = jax.devices()[:n_cores]
        assert len(devices) == n_cores
        if n_cores == 1:
            self._fn = jax.jit(_body, keep_unused=True)
        else:
            mesh = Mesh(np.asarray(devices), ("core",))
            in_specs = (PartitionSpec("core"),) * (n_params + n_outs)
            out_specs = (PartitionSpec("core"),) * n_outs
            self._fn = jax.jit(
                shard_map(_body, mesh=mesh, in_specs=in_specs,
                          out_specs=out_specs, check_rep=False),
                keep_unused=True,
            )

    def prep_inputs(self, in_maps):
        per_core = [[np.asarray(m[n]) for n in self.in_names] for m in in_maps]
        if self.n_cores == 1:
            args = per_core[0]
        else:
            args = [
                np.concatenate([per_core[c][i] for c in range(self.n_cores)], 0)
                for i in range(self.n_params)
            ]
        zouts = []
        for (s, d) in self.out_shapes:
            full = (s[0] * self.n_cores,) + tuple(s[1:]) \
                if self.n_cores > 1 else s
            zouts.append(np.zeros(full, d))
        return args + zouts

    def run(self, args):
        import jax
        outs = self._fn(*args)
        jax.block_until_ready(outs)
        return outs
